# revision 28
# baseline (speedup 1.0000x reference)
"""GAT conv layer on 8 TRN2 NeuronCores.

Row-parallel sharding: core c owns output rows [c*R, (c+1)*R).  Each core
receives its row-block of A pre-transposed (A^T: [N, R], fp8 {0,1}) plus
replicated X^T / W (bf16 hi/lo split for score accuracy).

Math (per head h, with s_ij = a_i + b_j, F = exp(leakyrelu(s, 0.2))):
  s > 0:  F = e^s     = g_i * h_j   (g = e^a, h = e^b)
  s <= 0: F = e^0.2s  = p_i * q_j   (p = e^0.2a, q = e^0.2b)
  M1 = A^T o (s > 0)  (computed in [j, i] layout, bf16 {0,1})
  num_i = g_i*(M1 @ h.f)_i + p_i*((A-M1) @ q.f)_i ;  Z same with f->1
  out = elu(num / Z), heads concatenated.
(A-M1)@qf is computed as A@qf - M1@qf via separate PSUM regions, so M2 is
never materialized.  exp is only ever applied to length-N vectors.

The mask build alternates between two engine paths so DVE and ACT share
the N^2-scale work:
  path A (DVE): c = (a_i > -b_j)  [TS is_gt],  m1 = c * at  [TT mult]
  path B (ACT): w = relu(G*(a_i + b_j)) with G=1e4, m1 = min(w, at) [TT]
(at is {0,1} bf16, DMA-cast per tile from the fp8 HBM copy; it is not
kept resident - each j-tile is streamed once per sweep.)  The rhs
[h*[f|1] | q*[f|1]] is precomputed per (head, tile) contiguously, so
phase 2 runs 3 matmuls per (tile, row-slice) instead of 5.
"""

import numpy as np
import ml_dtypes

import concourse.bass as bass
import concourse.mybir as mybir
import concourse.tile as tile
from concourse.bass_utils import run_bass_kernel_spmd

BF16 = ml_dtypes.bfloat16
FP8 = ml_dtypes.float8_e4m3
F32 = mybir.dt.float32
BF = mybir.dt.bfloat16
F8 = mybir.dt.float8e4

N, F_IN, UNITS, HEADS = 8192, 256, 64, 4
NCORES = 8
GAMMA = 16384.0


class PatchedTileContext(tile.TileContext):
    # This neuronxcc build rejects instructions carrying more than ONE sem
    # wait ("Too many sync wait commands" in setupSyncWait).  Split extra
    # waits onto InstEventSemaphore wait-carriers on the same engine,
    # committed immediately before the instruction (engine FIFO order makes
    # them blocking).
    def _commit_instruction(self, inst, lazy_reg_writes=True):
        si = inst.sync_info
        if si is not None and len(si.on_wait) > 1:
            waits = list(si.on_wait)
            for w in waits[:-1]:
                carrier = mybir.InstEventSemaphore(
                    name=self.nc.get_next_instruction_name(),
                    ins=[],
                    outs=[],
                    engine=inst.engine,
                    sync_info=mybir.SyncInfo(on_wait=[w], on_update=[]),
                )
                super()._commit_instruction(carrier, lazy_reg_writes)
            inst.sync_info = mybir.SyncInfo(
                on_wait=waits[-1:], on_update=list(si.on_update)
            )
        return super()._commit_instruction(inst, lazy_reg_writes)

    # Same issue for the final drain: put its waits one-per-instruction on
    # wait-carriers, then a wait-free drain; the all-engine barrier after
    # preserves ordering.
    def _drain_and_barrier(self, tick_clock, wait_clock):
        scratch = self.nc._final_wait_scratch
        first = self.nc.vector.memset(scratch[:, 0:1], 0.0)
        wait_clock.add_sem_waits(
            first.ins, tile.ScopedClock({None: tick_clock.global_clock})
        )
        si = first.ins.sync_info
        waits = list(si.on_wait) if si is not None else []
        if len(waits) > 1:
            first.ins.sync_info = mybir.SyncInfo(
                on_wait=waits[:1], on_update=list(si.on_update)
            )
            for i in range(1, len(waits)):
                extra = self.nc.vector.memset(scratch[:, i % 31 + 1 : i % 31 + 2], 0.0)
                extra.ins.sync_info = mybir.SyncInfo(
                    on_wait=waits[i : i + 1], on_update=[]
                )
        self.nc.sync.drain()
        self.nc.all_engine_barrier()
        assert self.sems is not None
        popped = self.nc._tile_sem_poison_stack.pop()
        assert popped is self._sem_poison
        self.nc.clear_and_free_semaphores(list(self.sems.allocated().values()))
        self.nc.all_engine_barrier()


def build_kernel(n=N, r=N // NCORES, f_in=F_IN, units=UNITS, heads=HEADS,
                 num_devices=NCORES):
    """Build the per-core SPMD graph.  Returns the Bass object."""
    assert n % 128 == 0 and r % 128 == 0 and f_in % 128 == 0
    nt = n // 128          # j tiles
    nk = f_in // 128       # contraction tiles for feats
    nslice = r // 128      # output row slices (PSUM groups)
    wcols = heads * units + heads          # feats cols + b cols
    uz = units + 1                         # [feats | ones] rhs cols per branch
    alu = mybir.AluOpType
    act = mybir.ActivationFunctionType

    nc = bass.Bass("TRN2", target_bir_lowering=False, debug=False,
                   num_devices=num_devices)
    nc._final_wait_scratch = nc.alloc_sbuf_tensor(
        "final_wait_scratch", [128, 32], F32).ap()

    at_d = nc.dram_tensor("AT", [n, r], BF, kind="ExternalInput").ap()
    xt_hi_d = nc.dram_tensor("XT_hi", [f_in, n], BF, kind="ExternalInput").ap()
    xt_lo_d = nc.dram_tensor("XT_lo", [f_in, n], BF, kind="ExternalInput").ap()
    xrt_hi_d = nc.dram_tensor("XRT_hi", [f_in, r], BF, kind="ExternalInput").ap()
    xrt_lo_d = nc.dram_tensor("XRT_lo", [f_in, r], BF, kind="ExternalInput").ap()
    w_hi_d = nc.dram_tensor("W_hi", [f_in, wcols], BF, kind="ExternalInput").ap()
    w_lo_d = nc.dram_tensor("W_lo", [f_in, wcols], BF, kind="ExternalInput").ap()
    wv_hi_d = nc.dram_tensor("WV_hi", [f_in, heads], BF, kind="ExternalInput").ap()
    wv_lo_d = nc.dram_tensor("WV_lo", [f_in, heads], BF, kind="ExternalInput").ap()
    eye_d = nc.dram_tensor("EYE", [128, 128], F32, kind="ExternalInput").ap()
    out_d = nc.dram_tensor("out", [r, heads * units], F32,
                           kind="ExternalOutput").ap()

    with PatchedTileContext(nc) as tc:
        with tc.tile_pool(name="persist", bufs=1) as persist:
            # ---------- persistent tiles ----------
            # rhs_all[:, h, t, 0, :] = h_j * [feats_h | 1]
            # rhs_all[:, h, t, 1, :] = q_j * [feats_h | 1]
            rhs = persist.tile([128, heads, nt, 2, uz], BF, name="rhs", tag="rhs")
            b_sb = persist.tile([128, nt, heads], F32, name="b_sb", tag="b_sb")
            gb_sb = persist.tile([128, nt, heads], F32, name="gb_sb", tag="gb_sb")
            h_sb = persist.tile([128, nt, heads], BF, name="h_sb", tag="h_sb")
            q_sb = persist.tile([128, nt, heads], BF, name="q_sb", tag="q_sb")
            g_sb = persist.tile([128, nslice, heads], F32, name="g_sb", tag="g_sb")
            p_sb = persist.tile([128, nslice, heads], F32, name="p_sb", tag="p_sb")
            a_sb = [persist.tile([1, r], F32, name=f"a_sb{h}", tag=f"a_sb{h}")
                    for h in range(heads)]
            abc = [persist.tile([128, r], BF, name=f"abc{h}", tag=f"abc{h}")
                   for h in range(heads)]
            eye = persist.tile([1, 1], F32, name="eye", tag="eye")
            out_sb = persist.tile([128, nslice, 2, units], F32, name="osb",
                                  tag="osb")
            nc.gpsimd.dma_start(eye[:], eye_d[0:1, 0:1])

            # prefetch the first A^T pairs during phase 1 (sync engine, no
            # dependencies) so phase 2 masks can start right away
            at2_pre = at_d.rearrange("(p q x) c -> p q x c", q=2, x=128)
            atpre = []
            for pp in range(1):
                tl = persist.tile([128, 2, r], BF, name=f"atp{pp}",
                                  tag=f"atp{pp}")
                nc.sync.dma_start(
                    tl[:], at2_pre[pp : pp + 1, :, :, :].rearrange(
                        "p q x c -> x (p q) c"))
                atpre.append(tl)

            # ---------- phase 1: feats / a / b ----------
            with (
                tc.tile_pool(name="ph1", bufs=1) as ph1,
                tc.tile_pool(name="ph1_psum", bufs=4, space="PSUM") as ph1_psum,
                tc.tile_pool(name="ph1_psum2", bufs=1, space="PSUM") as ph1_psum2,
            ):
                xt_hi = [ph1.tile([128, n], BF, name=f"xth{k}", tag=f"xth{k}") for k in range(nk)]
                xt_lo = [ph1.tile([128, n], BF, name=f"xtl{k}", tag=f"xtl{k}") for k in range(nk)]
                xrt_hi = [ph1.tile([128, r], BF, name=f"xrh{k}", tag=f"xrh{k}") for k in range(nk)]
                xrt_lo = [ph1.tile([128, r], BF, name=f"xrl{k}", tag=f"xrl{k}") for k in range(nk)]
                w_hi = [ph1.tile([128, wcols], BF, name=f"wh{k}", tag=f"wh{k}") for k in range(nk)]
                w_lo = [ph1.tile([128, wcols], BF, name=f"wl{k}", tag=f"wl{k}") for k in range(nk)]
                wv_hi = [ph1.tile([128, heads], BF, name=f"vh{k}", tag=f"vh{k}") for k in range(nk)]
                wv_lo = [ph1.tile([128, heads], BF, name=f"vl{k}", tag=f"vl{k}") for k in range(nk)]
                feats = ph1.tile([128, nt, heads, uz], BF, name="feats", tag="feats")
                # ones column (index `units` of each head block) survives
                # the strided drains below; it propagates h/q into the 65th
                # rhs columns when the broadcast-multiplies run.
                nc.vector.memset(feats[:], 1.0)
                for k in range(nk):
                    s = slice(k * 128, (k + 1) * 128)
                    nc.gpsimd.dma_start(w_hi[k][:], w_hi_d[s, :])
                    nc.gpsimd.dma_start(w_lo[k][:], w_lo_d[s, :])
                    nc.gpsimd.dma_start(wv_hi[k][:], wv_hi_d[s, :])
                    nc.gpsimd.dma_start(wv_lo[k][:], wv_lo_d[s, :])
                    for q0 in range(0, r, r // 2):
                        qs = slice(q0, q0 + r // 2)
                        nc.gpsimd.dma_start(xrt_hi[k][:, qs], xrt_hi_d[s, qs])
                        nc.gpsimd.dma_start(xrt_lo[k][:, qs], xrt_lo_d[s, qs])
                # n-chunked so feats tile t only waits for its slice
                for q0 in range(0, n, n // 8):
                    qs = slice(q0, q0 + n // 8)
                    for k in range(nk):
                        s = slice(k * 128, (k + 1) * 128)
                        nc.gpsimd.dma_start(xt_hi[k][:, qs], xt_hi_d[s, qs])
                        nc.gpsimd.dma_start(xt_lo[k][:, qs], xt_lo_d[s, qs])

                # a for this core's rows, one [1, r] row per head (base
                # partition 0 so it can feed PE as rhs)
                ab_chunk = min(512, r)
                for h in range(heads):
                    hh = slice(h, h + 1)
                    for half in range(r // ab_chunk):
                        hs = slice(half * ab_chunk, (half + 1) * ab_chunk)
                        pa = ph1_psum2.tile([1, ab_chunk], F32, name="pa",
                                            tag="pa", bufs=1)
                        for k in range(nk):
                            nc.tensor.matmul(pa[:], wv_hi[k][:, hh],
                                             xrt_hi[k][:, hs],
                                             start=(k == 0), stop=False)
                        for k in range(nk):
                            nc.tensor.matmul(pa[:], wv_lo[k][:, hh],
                                             xrt_hi[k][:, hs],
                                             start=False, stop=False)
                        for k in range(nk):
                            nc.tensor.matmul(pa[:], wv_hi[k][:, hh],
                                             xrt_lo[k][:, hs],
                                             start=False, stop=(k == nk - 1))
                        nc.scalar.copy(a_sb[h][0:1, hs], pa[:])

                # g/p in [i%128, islice, head] layout via PE transpose
                pg = ph1_psum2.tile([128, nslice, heads], F32, name="pg", tag="pg")
                n_tr = nslice * heads
                for sl in range(nslice):
                    for h in range(heads):
                        ti = sl * heads + h
                        nc.tensor.matmul(
                            pg[:, sl, h : h + 1],
                            a_sb[h][0:1, sl * 128 : (sl + 1) * 128],
                            eye[0:1, 0:1], is_transpose=True,
                            start=(ti == 0), stop=(ti == n_tr - 1))
                nc.scalar.activation(g_sb[:], pg[:], act.Exp)
                nc.scalar.activation(p_sb[:], pg[:], act.Exp, scale=0.2)

                # a broadcast to all partitions (bf16), per head: PE
                # outer-product ones[128] x a_row
                ones1 = ph1.tile([1, 128], F32, name="ones1", tag="ones1")
                nc.vector.memset(ones1[:], 1.0)
                for h in range(heads):
                    for half in range(r // ab_chunk):
                        hs = slice(half * ab_chunk, (half + 1) * ab_chunk)
                        pb = ph1_psum2.tile([128, ab_chunk], F32, name="pb",
                                            tag="pb", bufs=2)
                        nc.tensor.matmul(pb[:], ones1[:], a_sb[h][0:1, hs],
                                         start=True, stop=True)
                        nc.vector.tensor_copy(abc[h][:, hs], pb[:])
                bcol = slice(heads * units, wcols)
                # feats matmuls interleaved with the per-chunk rhs builds so
                # the DVE/ACT chunk work overlaps the feats loop (engines are
                # FIFO - emission order is the schedule)
                CH = min(8, nt)
                for c0 in range(0, nt, CH):
                    for t in range(c0, c0 + CH):
                        pf = ph1_psum.tile([128, wcols], F32, name="pf", tag="pf")
                        ts_ = slice(t * 128, (t + 1) * 128)
                        for k in range(nk):
                            nc.tensor.matmul(pf[:], xt_hi[k][:, ts_], w_hi[k][:],
                                             start=(k == 0), stop=False)
                        # hi/lo corrections, b columns only (score accuracy)
                        for k in range(nk):
                            nc.tensor.matmul(pf[:, bcol], xt_hi[k][:, ts_],
                                             w_lo[k][:, bcol], start=False,
                                             stop=False)
                        for k in range(nk):
                            nc.tensor.matmul(pf[:, bcol], xt_lo[k][:, ts_],
                                             w_hi[k][:, bcol], start=False,
                                             stop=(k == nk - 1))
                        nc.scalar.copy(feats[:, t, :, 0:units],
                                       pf[:, 0 : heads * units])
                        nc.scalar.copy(b_sb[:, t, :], pf[:, bcol])

                    # h = e^b, q = e^0.2b (bf16); rhs = h_j * [feats_h | 1],
                    # qp = q_j * [feats_h | 1]
                    cs = slice(c0, c0 + CH)
                    nc.scalar.activation(h_sb[:, cs, :], b_sb[:, cs, :], act.Exp)
                    nc.scalar.activation(q_sb[:, cs, :], b_sb[:, cs, :], act.Exp,
                                         scale=0.2)
                    nc.scalar.mul(gb_sb[:, cs, :], b_sb[:, cs, :], GAMMA)
                    reng = nc.vector
                    for h in range(heads):
                        fh = feats[:, cs, h, :]
                        hb = h_sb[:, cs, h : h + 1].broadcast_to([128, CH, uz])
                        qb = q_sb[:, cs, h : h + 1].broadcast_to([128, CH, uz])
                        reng.tensor_tensor(rhs[:, h, cs, 0, :], fh, hb,
                                           alu.mult)
                        reng.tensor_tensor(rhs[:, h, cs, 1, :], fh, qb,
                                           alu.mult)

            # ---------- phase 2: masked matmuls, 2 heads per sweep ----------
            # A^T streams from HBM (fp8) per (sweep, tile), DMA-cast to bf16
            # into a small ring; it is never resident.
            with (
                tc.tile_pool(name="psum_main", bufs=1, space="PSUM") as psum_main,
                tc.tile_pool(name="cm", bufs=2) as cm,
            ):
                # PREF pairs of the next sweep's masks are emitted before the
                # current sweep's epilogue, so the PE restarts immediately
                # after each PSUM bank is drained.  Tiles are processed in
                # pairs: one DMA dispatch and one mask-multiply per pair
                # (dispatch costs ~450ns of engine time; DVE ops have ~200ns
                # fixed overhead).
                PREF = 6
                npair = nt // 2
                at2_d = at_d.rearrange("(p q x) c -> p q x c", q=2, x=128)

                def emit_masks(sw, pair):
                    """DMA a pair of A^T tiles and build both heads' masks.
                    Returns (atb, [m1_h0, m1_h1]) with shapes [128, 2, r]."""
                    hp = (2 * sw, 2 * sw + 1)
                    if sw == 0 and pair < len(atpre):
                        atb = atpre[pair]
                    else:
                        atb = cm.tile([128, 2, r], BF, name="atb", tag="atb",
                                      bufs=2 + PREF)
                        nc.sync.dma_start(
                            atb[:], at2_d[pair : pair + 1, :, :, :].rearrange(
                                "p q x c -> x (p q) c"))
                    m1s = []
                    for hi_, h in enumerate(hp):
                        m1 = cm.tile([128, 2, r], BF, name="m1", tag="m1",
                                     bufs=3 + PREF)
                        # The (pair, head) units are spread over three paths
                        # so DVE, ACT and GpSimd share the N^2-scale compare
                        # work.  Ramp/boundary pairs go to DVE (idle there).
                        if pair < 6:
                            path = "A"
                        elif hi_ == 0:
                            path = "A" if pair % 4 < 2 else "B"
                        else:
                            path = "B"
                        if path == "A":
                            # DVE compares then one DVE mask-mult
                            c = cm.tile([128, 2, r], BF, name="c", tag="c",
                                        bufs=3)
                            for tp in range(2):
                                t = 2 * pair + tp
                                nc.vector.tensor_scalar(
                                    c[:, tp, :], abc[h][:],
                                    b_sb[:, t, h : h + 1], 0.0,
                                    alu.add, alu.is_gt)
                            nc.vector.tensor_tensor(m1[:], c[:], atb[:],
                                                    alu.mult)
                        else:
                            # path B: ACT steep-relus then one DVE min
                            w1 = cm.tile([128, 2, r], BF, name="w1", tag="w1",
                                         bufs=3)
                            for tp in range(2):
                                t = 2 * pair + tp
                                nc.scalar.activation(
                                    w1[:, tp, :], abc[h][:], act.Relu,
                                    bias=gb_sb[:, t, h : h + 1], scale=GAMMA)
                            nc.vector.tensor_tensor(m1[:], w1[:], atb[:],
                                                    alu.min)
                        m1s.append(m1)
                    return atb, m1s

                def emit_mms(sw, pair, ps, masks):
                    hp = (2 * sw, 2 * sw + 1)
                    atb, m1s = masks
                    for tp in range(2):
                        t = 2 * pair + tp
                        for sl in range(nslice):
                            ssl = slice(sl * 128, (sl + 1) * 128)
                            # one zero-region (bank) per ps[sl]: start only on
                            # the first matmul of t==0, stop only on the last
                            # of t==nt-1
                            nc.tensor.matmul(
                                ps[sl][:, 0, :, :],
                                m1s[0][:, tp, ssl], rhs[:, hp[0], t, :, :],
                                start=(t == 0), stop=False)
                            nc.tensor.matmul(
                                ps[sl][:, 1, :, :],
                                m1s[1][:, tp, ssl], rhs[:, hp[1], t, :, :],
                                start=False, stop=False)
                            nc.tensor.matmul(
                                ps[sl][:, 2, :, :],
                                atb[:, tp, ssl],
                                rhs[:, hp[0] : hp[0] + 2, t, 1, :],
                                start=False, stop=(t == nt - 1))

                pref_masks = []
                for sw in range(2):
                    hp = (2 * sw, 2 * sw + 1)
                    ps = [psum_main.tile([128, 3, 2, uz], F32, name=f"ps{sl}",
                                         tag=f"ps{sl}")
                          for sl in range(nslice)]
                    # per islice psum layout: [h0: 2*uz | h1: 2*uz | C: 2*uz]
                    for pair in range(npair):
                        if pair < len(pref_masks):
                            masks = pref_masks[pair]
                        else:
                            masks = emit_masks(sw, pair)
                        emit_mms(sw, pair, ps, masks)

                    # ---------- epilogue for this sweep (head-paired),
                    # interleaved with next sweep's mask prefetch ----------
                    for sl in range(nslice):
                        if sw == 0 and sl < PREF:
                            pref_masks.append(emit_masks(sw + 1, sl))
                        e1 = cm.tile([128, 2, uz], F32, name="e1", tag="e1", bufs=2)
                        e2 = cm.tile([128, 2, uz], F32, name="e2", tag="e2", bufs=2)
                        e3 = cm.tile([128, 2, uz], F32, name="e3", tag="e3", bufs=2)
                        for hi_, h in enumerate(hp):
                            ga = g_sb[:, sl, h : h + 1]
                            pa_ = p_sb[:, sl, h : h + 1]
                            # e1 = g*A, e2 = p*B, e3 = p*C  (one PSUM operand
                            # per instruction; t2 on DVE splits the load)
                            nc.scalar.activation(e1[:, hi_, :],
                                                 ps[sl][:, hi_, 0, :],
                                                 act.Copy, scale=ga)
                            if sl % 2 == 0:
                                nc.vector.tensor_scalar(e2[:, hi_, :],
                                                        ps[sl][:, hi_, 1, :],
                                                        pa_, None, alu.mult)
                            else:
                                nc.scalar.activation(e2[:, hi_, :],
                                                     ps[sl][:, hi_, 1, :],
                                                     act.Copy, scale=pa_)
                            nc.scalar.activation(e3[:, hi_, :],
                                                 ps[sl][:, 2, hi_, :],
                                                 act.Copy, scale=pa_)
                        veng = nc.gpsimd if sl % 3 == 2 else nc.vector
                        t4 = cm.tile([128, 2, uz], F32, name="t4", tag="t4", bufs=2)
                        veng.tensor_tensor(t4[:], e3[:], e2[:],
                                           alu.subtract)
                        nz = cm.tile([128, 2, uz], F32, name="nz", tag="nz", bufs=2)
                        veng.tensor_tensor(nz[:], e1[:], t4[:], alu.add)
                        rz = cm.tile([128, 2, 1], F32, name="rz", tag="rz", bufs=2)
                        nc.vector.reciprocal(rz[:], nz[:, :, units : units + 1])
                        o = cm.tile([128, 2, units], F32, name="o", tag="o", bufs=2)
                        veng.tensor_tensor(o[:], nz[:, :, 0:units],
                                           rz[:].broadcast_to([128, 2, units]),
                                           alu.mult)
                        # elu: out = (relu(o) - 1) + e^min(o,0), with
                        # e^min(o,0) = exp(-relu(-o)) so both steps run on ACT
                        xm = cm.tile([128, 2, units], F32, name="xm", tag="xm", bufs=2)
                        nc.scalar.activation(xm[:], o[:], act.Relu, scale=-1.0)
                        ex = cm.tile([128, 2, units], F32, name="ex", tag="ex", bufs=2)
                        nc.scalar.activation(ex[:], xm[:], act.Exp, scale=-1.0)
                        d = cm.tile([128, 2, units], F32, name="d", tag="d", bufs=2)
                        veng.tensor_scalar(d[:], o[:], 0.0, -1.0,
                                           alu.max, alu.add)
                        veng.tensor_tensor(
                            out_sb[:, sl, :, :], d[:], ex[:], alu.add)

                    # out rows i = sl*128 + p, cols [2*sw*units, (2*sw+2)*units)
                    dst = out_d[:, 2 * sw * units : (2 * sw + 2) * units]
                    dst = dst.rearrange("(s p) u -> p s u", p=128)
                    for sl in range(nslice):
                        nc.sync.dma_start(dst[:, sl : sl + 1, :],
                                          out_sb[:, sl : sl + 1, :, :])

    return nc


_CACHE = {}


def _get_nc():
    if "nc" not in _CACHE:
        _CACHE["nc"] = build_kernel()
    return _CACHE["nc"]


def _split_bf16(x):
    hi = np.asarray(x, dtype=BF16)
    lo = np.asarray(x - np.asarray(hi, dtype=np.float32), dtype=BF16)
    return hi, lo


def prep_in_maps(X, A, W, attn_self, attn_neigh, ncores=NCORES):
    X = np.asarray(X, dtype=np.float32)
    A = np.asarray(A, dtype=np.float32)
    W = np.asarray(W, dtype=np.float32)
    heads, f_in, units = W.shape
    n = X.shape[0]
    r = n // ncores

    # W_full: [F_IN, H*U feats cols (h-major) | H b-cols]
    w_full = np.zeros((f_in, heads * units + heads), dtype=np.float32)
    for h in range(heads):
        w_full[:, h * units : (h + 1) * units] = W[h]
        w_full[:, heads * units + h] = W[h] @ np.asarray(attn_neigh[h],
                                                        dtype=np.float32)
    wv = np.stack([W[h] @ np.asarray(attn_self[h], dtype=np.float32)
                   for h in range(heads)], axis=1)       # [F, H]

    xt = np.ascontiguousarray(X.T)                       # [F, N]
    xt_hi, xt_lo = _split_bf16(xt)
    w_hi, w_lo = _split_bf16(w_full)
    wv_hi, wv_lo = _split_bf16(wv)
    eye = np.eye(128, dtype=np.float32)

    in_maps = []
    for c in range(ncores):
        rows = slice(c * r, (c + 1) * r)
        in_maps.append({
            "AT": np.ascontiguousarray(A[rows, :].T).astype(BF16),
            "XT_hi": xt_hi, "XT_lo": xt_lo,
            "XRT_hi": np.ascontiguousarray(xt_hi[:, rows]),
            "XRT_lo": np.ascontiguousarray(xt_lo[:, rows]),
            "W_hi": w_hi, "W_lo": w_lo,
            "WV_hi": wv_hi, "WV_lo": wv_lo,
            "EYE": eye,
        })
    return in_maps


def kernel(X, A, W, attn_self, attn_neigh, _trace=False):
    in_maps = prep_in_maps(X, A, W, attn_self, attn_neigh)
    nc = _get_nc()
    res = run_bass_kernel_spmd(nc, in_maps, list(range(NCORES)), trace=_trace)
    kernel.last_exec_time_ns = res.exec_time_ns
    out = np.concatenate([res.results[c]["out"] for c in range(NCORES)], axis=0)
    return out.astype(np.float32)


kernel.last_exec_time_ns = None
